# revision 29
# baseline (speedup 1.0000x reference)
"""CopyTokenDecoder Trainium2 kernel.

Sharding: data-parallel over batch B=8 -> one NeuronCore per batch element.
Each core runs the full per-batch pipeline: single-head attention front-end,
gating, FFN, the [T,D]x[D,V] output projection with local softmax over the
full vocab, the copy-mechanism scatter-add (realized as small matmuls against
a host-built routing one-hot), and the final log.

Per-core layouts (P = 128 partitions):
  feature-major  [d_lo(128), d_hi, t]   for matmul operands
  token-major    [t_lo(128), t_hi, d]   for layernorms / row-wise scaling
The s (memory) axis is host-side sorted by copy-token vocab bucket and padded
to a 64-window x WSLOT slot grid so the scatter becomes, per 512-wide vocab
tile, one K=WSLOT matmul (exp_scores-window x one-hot window). Padded slots
are masked to -1e30 pre-softmax so they contribute exp()=0 everywhere.
"""

from contextlib import ExitStack

import numpy as np
import ml_dtypes

import concourse.tile as tile
from concourse import bacc, mybir
from concourse.bass_utils import run_bass_kernel_spmd
from concourse.masks import make_identity

F32 = mybir.dt.float32
BF16 = mybir.dt.bfloat16
F8 = mybir.dt.float8e4
AF = mybir.ActivationFunctionType
OP = mybir.AluOpType
PM = mybir.MatmulPerfMode
BF = ml_dtypes.bfloat16
F8NP = ml_dtypes.float8_e4m3

T, B, S, D, F, V = 256, 8, 512, 512, 2048, 32000
P = 128
DSCALE = float(D) ** -0.5
NEG = -1.0e30
TT = 2                      # t-tiles of 128
NWIN = 63                   # 512-wide vocab windows (last covers 256)
NCHUNK = 16                 # vocab chunks of 2048 (last 1280)
CHUNK = 2048
EPS_LN = 1e-5
EPS_LOG = 1e-12
SW = 64.0                   # fp8 scale on W_emb
SX = 8.0                    # fp8 scale on x2 (folded into ln2 gain)
SE = 32.0                   # scale on exp(score) (via Exp bias)
LN_SE = float(np.log(SE))
LN_E = float(np.log(8.0))   # E' = 8*exp(logit)

_CACHE = {}


def _subwidths(c):
    if c < NCHUNK - 1:
        return [512, 512, 512, 512]
    return [512, 512, 256]


def _build(wslot):
    sp = 64 * wslot           # padded slot count (s' axis)
    nhi = sp // P             # s'-outer size
    wpb = P // wslot          # windows per 128-partition block

    nc = bacc.Bacc("TRN2", target_bir_lowering=False, debug=False,
                   enable_asserts=False, num_devices=B)

    def din(name, shape, dt):
        return nc.dram_tensor(name, shape, dt, kind="ExternalInput").ap()

    # per-core tensors
    outsT_d = din("outsT", [D, T], F8)
    outs_tok_d = din("outs_tok", [T, D], F32)
    memT_d = din("memT", [D, S], F8)
    maskrow_d = din("maskrow", [1, S], BF16)
    pmat_d = din("pmat", [S, sp], BF16)
    wembC_d = din("wembC", [D, sp], F8)
    onehot_d = din("onehot", [P, nhi * 512], BF16)
    # shared weights
    wqT_d = din("wqT", [D, D], F8)
    wkT_d = din("wkT", [D, D], F8)
    wvT_d = din("wvT", [D, D], F8)
    woT_d = din("woT", [D, D], F8)
    w1T_d = din("w1T", [D, F], F8)
    w2T_d = din("w2T", [F, D], F8)
    wembW_d = din("wembW", [NCHUNK, P, 4 * 4 * 512], F8)
    bq_d = din("bq_c", [P, 4], F32)
    bk_d = din("bk_c", [P, 4], F32)
    bvrow_d = din("bv_row", [P, D], F32)
    bo_tok_d = din("bo_tok", [P, D], F32)
    b1_d = din("b1_c", [P, 16], F32)
    b2_d = din("b2_c", [P, 4], F32)
    g1_d = din("g1_tok", [P, D], F32)
    b1g_d = din("b1g_tok", [P, D], F32)
    g2_d = din("g2_tok", [P, D], F32)
    b2g_d = din("b2g_tok", [P, D], F32)
    wdd_d = din("wd_diff_tok", [P, 2 * D], F32)
    bdd_d = din("bddiff", [P, 1], F32)
    ones_d = din("ones_row", [1, T], BF16)

    out_d = nc.dram_tensor("out", [T, V], BF16, kind="ExternalOutput").ap()
    out_r = out_d.rearrange("(th tl) v -> tl th v", tl=P)

    r3 = lambda ap, inner: ap.rearrange("(hi lo) x -> lo hi x", lo=P)

    with tile.TileContext(nc) as tc, ExitStack() as octx:
        cpool = octx.enter_context(tc.tile_pool(name="cpool", bufs=1))
        # ---- persistent tiles (live through pass A/B) ----
        onehot = cpool.tile([P, nhi, 512], BF16, tag="onehot")
        nc.sync.dma_start(onehot[:], onehot_d.rearrange("p (hi v) -> p hi v", v=512))
        exp_c = cpool.tile([P, 4, T], BF16, tag="exp_c")
        x2T = cpool.tile([P, 4, T], F8, tag="x2T")
        ident_f = cpool.tile([P, P], F32, tag="ident_f")
        make_identity(nc, ident_f[:])
        ident_b = cpool.tile([P, P], BF16, tag="ident_b")
        nc.vector.tensor_copy(ident_b[:], ident_f[:])
        rr = cpool.tile([P, TT], F32, tag="rr")          # 1/sum_s exp(scores)
        cg = cpool.tile([P, TT], F32, tag="cg")          # copy gate
        gg = cpool.tile([P, TT], F32, tag="gg")          # gen gate
        sv_parts = cpool.tile([P, 2 * NCHUNK], F32, tag="sv_parts")
        svs = cpool.tile([P, TT], F32, tag="svs")
        beta = cpool.tile([P, TT], F32, tag="beta")      # bf16-rounded, as f32
        slog = cpool.tile([P, TT], F32, tag="slog")      # final log scale
        onesr = cpool.tile([1, T], BF16, tag="onesr")
        nc.sync.dma_start(onesr[:], ones_d[:])
        pmat = cpool.tile([P, 4, sp], BF16, tag="pmat")
        nc.sync.dma_start(pmat[:], r3(pmat_d, sp))
        wembC = cpool.tile([P, 4, sp], F8, tag="wembC")
        nc.sync.dma_start(wembC[:], r3(wembC_d, sp))
        maskrow = cpool.tile([1, S], BF16, tag="maskrow")
        nc.sync.dma_start(maskrow[:], maskrow_d[:])
        eps_ln_c = cpool.tile([P, 1], F32, tag="eps_ln_c")
        nc.gpsimd.memset(eps_ln_c[:], EPS_LN)
        eps_log_c = cpool.tile([P, 1], F32, tag="eps_log_c")
        nc.gpsimd.memset(eps_log_c[:], EPS_LOG)
        lnse_c = cpool.tile([P, 1], F32, tag="lnse_c")
        nc.gpsimd.memset(lnse_c[:], LN_SE)
        lne_c = cpool.tile([P, 1], F32, tag="lne_c")
        nc.gpsimd.memset(lne_c[:], LN_E)
        one_c = cpool.tile([P, 1], F32, tag="one_c")
        nc.gpsimd.memset(one_c[:], 1.0)

        # ================= front-end (staged scoped pools) =================
        fctx = ExitStack()
        fe = fctx.enter_context(tc.tile_pool(name="fe", bufs=1))

        def load(pool, dram_ap, shape, tag):
            t_ = pool.tile(shape, dram_ap.dtype, tag=tag, name=tag)
            nc.sync.dma_start(t_[:], dram_ap)
            return t_

        # persists across both front-end stages
        outs_tok = load(fe, outs_tok_d.rearrange("(th tl) d -> tl th d", tl=P),
                        [P, TT, D], "outs_tok")
        bo_tok = load(fe, bo_tok_d, [P, D], "bo_tok")
        g1_tok = load(fe, g1_d, [P, D], "g1_tok")
        b1g_tok = load(fe, b1g_d, [P, D], "b1g_tok")
        g2_tok = load(fe, g2_d, [P, D], "g2_tok")
        b2g_tok = load(fe, b2g_d, [P, D], "b2g_tok")
        wd_diff = load(fe, wdd_d, [P, 2 * D], "wd_diff")
        bddiff = load(fe, bdd_d, [P, 1], "bddiff")
        attn_tok = fe.tile([P, TT, D], F32, tag="attn_tok")

        def layer_norm(pool, scr_pool, dst, src_ap, g_t, b_t, nm):
            """dst[:] = LN(src_ap) * g + b   (token-major [P, D] slices)"""
            mu = pool.tile([P, 1], F32, tag=f"mu_{nm}", name=f"mu_{nm}")
            nc.vector.reduce_sum(mu[:], src_ap, axis=mybir.AxisListType.X)
            nc.vector.tensor_scalar(out=mu[:], in0=mu[:], scalar1=1.0 / D,
                                    scalar2=None, op0=OP.mult)
            xc = scr_pool.tile([P, D], F32, tag="ln_xc", name="ln_xc")
            nc.vector.tensor_scalar(out=xc[:], in0=src_ap, scalar1=mu[:],
                                    scalar2=None, op0=OP.subtract)
            scr = scr_pool.tile([P, D], F32, tag="ln_scr", name="ln_scr")
            ss = pool.tile([P, 1], F32, tag=f"ss_{nm}", name=f"ss_{nm}")
            nc.scalar.activation(scr[:], xc[:], AF.Square, accum_out=ss[:])
            std = pool.tile([P, 1], F32, tag=f"std_{nm}", name=f"std_{nm}")
            nc.scalar.activation(std[:], ss[:], AF.Sqrt, bias=eps_ln_c[:, :1],
                                 scale=1.0 / D)
            rstd = pool.tile([P, 1], F32, tag=f"rstd_{nm}", name=f"rstd_{nm}")
            nc.vector.reciprocal(rstd[:], std[:])
            nc.vector.scalar_tensor_tensor(out=dst, in0=xc[:], scalar=rstd[:],
                                           in1=g_t[:], op0=OP.mult, op1=OP.mult)
            nc.vector.tensor_add(dst, dst, b_t[:])

        # ---------------- stage A: attention ----------------
        with ExitStack() as actx:
            fa = actx.enter_context(tc.tile_pool(name="fa", bufs=1))
            fad = actx.enter_context(tc.tile_pool(name="fad", bufs=2))
            fp = actx.enter_context(tc.tile_pool(name="fp", bufs=6, space="PSUM"))
            fp5 = actx.enter_context(tc.tile_pool(name="fp5", bufs=2, space="PSUM"))

            outsT = load(fa, r3(outsT_d, T), [P, 4, T], "outsT")
            memT = load(fa, r3(memT_d, S), [P, 4, S], "memT")

            wqT = load(fa, r3(wqT_d, D), [P, 4, D], "wqT")
            wkT = load(fa, r3(wkT_d, D), [P, 4, D], "wkT")
            wvT = load(fa, r3(wvT_d, D), [P, 4, D], "wvT")
            woT = load(fa, r3(woT_d, D), [P, 4, D], "woT")
            bq_c = load(fa, bq_d, [P, 4], "bq_c")
            bk_c = load(fa, bk_d, [P, 4], "bk_c")
            bvrow_t = load(fa, bvrow_d, [P, D], "bv_row")

            # PE warm-up: ~4us of junk matmuls gated only on on-chip data
            # (identity/memset) so they run while the input DMAs land and the
            # HAM clock-gate reaches 8/8 before the real matmuls start.
            wu = fa.tile([P, 512], BF16, tag="wu")
            nc.gpsimd.memset(wu[:], 0.0)
            wu_ps = fp5.tile([P, 512], F32, tag="ps512", space="PSUM")
            for i in range(20):
                nc.tensor.matmul(wu_ps[:], lhsT=ident_b[:], rhs=wu[:],
                                 start=(i == 0), stop=(i == 19))

            # q/k projections (feature-major, s compact & host-sorted)
            qT = fa.tile([P, 4, T], F8, tag="qT")
            for ho in range(4):
                ps = fp.tile([P, T], F32, tag="ps256", space="PSUM")
                for kp in range(2):
                    nc.tensor.matmul(ps[:],
                                     lhsT=wqT[:, 2 * kp:2 * kp + 2,
                                              ho * P:(ho + 1) * P],
                                     rhs=outsT[:, 2 * kp:2 * kp + 2, :],
                                     start=(kp == 0), stop=(kp == 1),
                                     perf_mode=PM.DoubleRow)
                # qT' = 64*q = (ps + 64*bq) * DSCALE   (ps = 64*outs@Wq.T)
                nc.vector.tensor_scalar(out=qT[:, ho, :], in0=ps[:],
                                        scalar1=bq_c[:, ho:ho + 1],
                                        scalar2=DSCALE, op0=OP.add, op1=OP.mult)
            kT = fa.tile([P, 4, S], F8, tag="kT")
            for ho in range(4):
                ps = fp5.tile([P, 512], F32, tag="ps512", space="PSUM")
                for kp in range(2):
                    nc.tensor.matmul(ps[:],
                                     lhsT=wkT[:, 2 * kp:2 * kp + 2,
                                              ho * P:(ho + 1) * P],
                                     rhs=memT[:, 2 * kp:2 * kp + 2, :],
                                     start=(kp == 0), stop=(kp == 1),
                                     perf_mode=PM.DoubleRow)
                # kT' = 16*k = (ps + 64*bk) / 4
                nc.vector.tensor_scalar(out=kT[:, ho, :], in0=ps[:],
                                        scalar1=bk_c[:, ho:ho + 1],
                                        scalar2=0.25, op0=OP.add, op1=OP.mult)
            # v (s-major)
            v_sb = fa.tile([P, 4, D], BF16, tag="v_sb")
            for sc in range(4):
                ps = fp5.tile([P, 512], F32, tag="ps512", space="PSUM")
                for kp in range(2):
                    nc.tensor.matmul(ps[:],
                                     lhsT=memT[:, 2 * kp:2 * kp + 2,
                                               sc * P:(sc + 1) * P],
                                     rhs=wvT[:, 2 * kp:2 * kp + 2, :],
                                     start=(kp == 0), stop=(kp == 1),
                                     perf_mode=PM.DoubleRow)
                # v = ps/64  (bv folded host-side as 64*bv via bvrow_t)
                nc.vector.scalar_tensor_tensor(out=v_sb[:, sc, :], in0=ps[:],
                                               scalar=1.0 / 64.0,
                                               in1=bvrow_t[:], op0=OP.mult,
                                               op1=OP.add)

            # scoresT -> exp_c (s-major, compact)
            for sc in range(4):
                ps = fp.tile([P, T], F32, tag="ps256", space="PSUM")
                for kp in range(2):
                    nc.tensor.matmul(ps[:],
                                     lhsT=kT[:, 2 * kp:2 * kp + 2,
                                             sc * P:(sc + 1) * P],
                                     rhs=qT[:, 2 * kp:2 * kp + 2, :],
                                     start=(kp == 0), stop=False,
                                     perf_mode=PM.DoubleRow)
                nc.tensor.matmul(ps[:], lhsT=maskrow[:1, sc * P:(sc + 1) * P],
                                 rhs=onesr[:1, :], start=False, stop=True)
                nc.scalar.activation(exp_c[:, sc, :], ps[:], AF.Exp,
                                     scale=1.0 / 1024.0, bias=lnse_c[:, :1])

            # scores token-major: only for attention softmax row-sums
            ratt_parts = fa.tile([P, TT], F32, tag="ratt_parts")
            for tt in range(TT):
                ps = fp5.tile([P, 512], F32, tag="ps512", space="PSUM")
                for kp in range(2):
                    nc.tensor.matmul(ps[:],
                                     lhsT=qT[:, 2 * kp:2 * kp + 2,
                                             tt * P:(tt + 1) * P],
                                     rhs=kT[:, 2 * kp:2 * kp + 2, :],
                                     start=(kp == 0), stop=False,
                                     perf_mode=PM.DoubleRow)
                nc.tensor.matmul(ps[:], lhsT=onesr[:1, :P], rhs=maskrow[:1, :],
                                 start=False, stop=True)
                scr = fad.tile([P, 512], F32, tag="scr_ts", name="scr_ts")
                nc.scalar.activation(scr[:], ps[:], AF.Exp, scale=1.0 / 1024.0,
                                     bias=lnse_c[:, :1],
                                     accum_out=ratt_parts[:, tt:tt + 1])
            nc.vector.reciprocal(rr[:], ratt_parts[:])

            # attention value mix + output projection (feature-major)
            attnT = fa.tile([P, 4, T], F8, tag="attnT")
            for dc in range(4):
                ps = fp.tile([P, T], F32, tag="ps256", space="PSUM")
                for sc in range(4):
                    nc.tensor.matmul(ps[:], lhsT=v_sb[:, sc, dc * P:(dc + 1) * P],
                                     rhs=exp_c[:, sc, :], start=(sc == 0),
                                     stop=(sc == 3))
                # attnT' = ps/32 (keep fp8 in range; exp_c carries x32)
                if dc % 2 == 0:
                    nc.vector.tensor_scalar(out=attnT[:, dc, :], in0=ps[:],
                                            scalar1=1.0 / 32.0, scalar2=None,
                                            op0=OP.mult)
                else:
                    nc.scalar.mul(attnT[:, dc, :], ps[:], 1.0 / 32.0)
            attn_oT = fa.tile([P, 4, T], F32, tag="attn_oT")
            for ho in range(4):
                ps = fp.tile([P, T], F32, tag="ps256", space="PSUM")
                for kp in range(2):
                    nc.tensor.matmul(ps[:],
                                     lhsT=woT[:, 2 * kp:2 * kp + 2,
                                              ho * P:(ho + 1) * P],
                                     rhs=attnT[:, 2 * kp:2 * kp + 2, :],
                                     start=(kp == 0), stop=(kp == 1),
                                     perf_mode=PM.DoubleRow)
                # attn_oT = ps*32/64 (undo attnT'/32, descale 64*Wo)
                if ho % 2 == 0:
                    nc.vector.tensor_scalar(out=attn_oT[:, ho, :], in0=ps[:],
                                            scalar1=0.5, scalar2=None,
                                            op0=OP.mult)
                else:
                    nc.scalar.mul(attn_oT[:, ho, :], ps[:], 0.5)

            # transpose to token-major; normalize rows; add bo
            for tt in range(TT):
                for ho in range(4):
                    pst = fp.tile([P, T], F32, tag="ps256", space="PSUM")
                    nc.tensor.transpose(pst[:, :P],
                                        attn_oT[:, ho, tt * P:(tt + 1) * P],
                                        ident_f[:])
                    if ho % 2 == 0:
                        nc.vector.tensor_copy(
                            attn_tok[:, tt, ho * P:(ho + 1) * P], pst[:, :P])
                    else:
                        nc.scalar.copy(attn_tok[:, tt, ho * P:(ho + 1) * P],
                                       pst[:, :P])
            for tt in range(TT):
                nc.vector.scalar_tensor_tensor(
                    out=attn_tok[:, tt, :], in0=attn_tok[:, tt, :],
                    scalar=rr[:, tt:tt + 1], in1=bo_tok[:],
                    op0=OP.mult, op1=OP.add)

        # ---------------- stage B: gates + FFN ----------------
        with ExitStack() as bctx:
            fb = bctx.enter_context(tc.tile_pool(name="fb", bufs=1))
            fbd = bctx.enter_context(tc.tile_pool(name="fbd", bufs=2))
            fp = bctx.enter_context(tc.tile_pool(name="fp2", bufs=8, space="PSUM"))

            w1T = load(fb, r3(w1T_d, F), [P, 4, F], "w1T")
            w2T = load(fb, r3(w2T_d, D), [P, 16, D], "w2T")
            b1_c = load(fb, b1_d, [P, 16], "b1_c")
            b2_c = load(fb, b2_d, [P, 4], "b2_c")

            # gates from [outs ; LN(attn)] (softmax2 == sigmoid of logit diff)
            attn_n = fbd.tile([P, D], F32, tag="attn_n", name="attn_n")
            ld = fb.tile([P, TT], F32, tag="ld")
            for tt in range(TT):
                layer_norm(fb, fbd, attn_n[:], attn_tok[:, tt, :], g1_tok,
                           b1g_tok, "an")
                lda = fb.tile([P, 1], F32, tag="lda")
                ldb = fb.tile([P, 1], F32, tag="ldb")
                scr = fbd.tile([P, D], F32, tag="ld_scr", name="ld_scr")
                nc.vector.scalar_tensor_tensor(out=scr[:], in0=outs_tok[:, tt, :],
                                               scalar=1.0, in1=wd_diff[:, :D],
                                               op0=OP.mult, op1=OP.mult,
                                               accum_out=lda[:])
                scr2 = fbd.tile([P, D], F32, tag="ld_scr2", name="ld_scr2")
                nc.vector.scalar_tensor_tensor(out=scr2[:], in0=attn_n[:],
                                               scalar=1.0, in1=wd_diff[:, D:],
                                               op0=OP.mult, op1=OP.mult,
                                               accum_out=ldb[:])
                nc.vector.tensor_add(ld[:, tt:tt + 1], lda[:], ldb[:])
            nc.scalar.activation(cg[:], ld[:], AF.Sigmoid, bias=bddiff[:, :1])
            nc.vector.tensor_scalar(out=gg[:], in0=cg[:], scalar1=-1.0,
                                    scalar2=1.0, op0=OP.mult, op1=OP.add)

            # residual + LN1 -> x ; FFN ; LN2 -> x2 ; transpose -> x2T
            x_tok = fb.tile([P, TT, D], F32, tag="x_tok")
            for tt in range(TT):
                res = fbd.tile([P, D], F32, tag="res", name="res")
                nc.vector.tensor_add(res[:], outs_tok[:, tt, :],
                                     attn_tok[:, tt, :])
                layer_norm(fb, fbd, x_tok[:, tt, :], res[:], g1_tok, b1g_tok, "x")
            xT = fb.tile([P, 4, T], F8, tag="xT")
            for tt in range(TT):
                for k in range(4):
                    pst = fp.tile([P, T], F32, tag="ps256", space="PSUM")
                    nc.tensor.transpose(pst[:, :P],
                                        x_tok[:, tt, k * P:(k + 1) * P],
                                        ident_f[:])
                    if k % 2 == 0:
                        nc.vector.tensor_copy(xT[:, k, tt * P:(tt + 1) * P],
                                              pst[:, :P])
                    else:
                        nc.scalar.copy(xT[:, k, tt * P:(tt + 1) * P],
                                       pst[:, :P])
            h1T = fb.tile([P, 16, T], F8, tag="h1T")
            for fc in range(16):
                ps = fp.tile([P, T], F32, tag="ps256", space="PSUM")
                for kp in range(2):
                    nc.tensor.matmul(ps[:],
                                     lhsT=w1T[:, 2 * kp:2 * kp + 2,
                                              fc * P:(fc + 1) * P],
                                     rhs=xT[:, 2 * kp:2 * kp + 2, :],
                                     start=(kp == 0), stop=(kp == 1),
                                     perf_mode=PM.DoubleRow)
                # h1T' = relu(ps + 64*b1) = 64*h1   (64*W1 fp8)
                nc.vector.tensor_scalar(out=h1T[:, fc, :], in0=ps[:],
                                        scalar1=b1_c[:, fc:fc + 1], scalar2=0.0,
                                        op0=OP.add, op1=OP.max)
            hT = fb.tile([P, 4, T], F32, tag="hT")
            for ho in range(4):
                ps = fp.tile([P, T], F32, tag="ps256", space="PSUM")
                for kp in range(8):
                    nc.tensor.matmul(ps[:],
                                     lhsT=w2T[:, 2 * kp:2 * kp + 2,
                                              ho * P:(ho + 1) * P],
                                     rhs=h1T[:, 2 * kp:2 * kp + 2, :],
                                     start=(kp == 0), stop=(kp == 7),
                                     perf_mode=PM.DoubleRow)
                # h = ps/4096 + b2   (64*W2, 64*h1)
                nc.vector.tensor_scalar(out=hT[:, ho, :], in0=ps[:],
                                        scalar1=1.0 / 4096.0,
                                        scalar2=b2_c[:, ho:ho + 1],
                                        op0=OP.mult, op1=OP.add)
            h_tok = fb.tile([P, TT, D], F32, tag="h_tok")
            for tt in range(TT):
                for ho in range(4):
                    pst = fp.tile([P, T], F32, tag="ps256", space="PSUM")
                    nc.tensor.transpose(pst[:, :P],
                                        hT[:, ho, tt * P:(tt + 1) * P],
                                        ident_f[:])
                    if ho % 2 == 0:
                        nc.vector.tensor_copy(
                            h_tok[:, tt, ho * P:(ho + 1) * P], pst[:, :P])
                    else:
                        nc.scalar.copy(h_tok[:, tt, ho * P:(ho + 1) * P],
                                       pst[:, :P])
            x2_tok = fb.tile([P, TT, D], F32, tag="x2_tok")
            for tt in range(TT):
                layer_norm(fb, fbd, x2_tok[:, tt, :], h_tok[:, tt, :], g2_tok,
                           b2g_tok, "x2")
            for tt in range(TT):
                for k in range(4):
                    pst = fp.tile([P, T], F32, tag="ps256", space="PSUM")
                    nc.tensor.transpose(pst[:, :P],
                                        x2_tok[:, tt, k * P:(k + 1) * P],
                                        ident_f[:])
                    if k % 2 == 0:
                        nc.vector.tensor_copy(x2T[:, k, tt * P:(tt + 1) * P],
                                              pst[:, :P])
                    else:
                        nc.scalar.copy(x2T[:, k, tt * P:(tt + 1) * P],
                                       pst[:, :P])

        fctx.close()

        # ================= vocab passes =================
        # out = logit + c_row + Delta, Delta = log(1 + copy/gen) scattered from
        # the compact slot grid. Pass A computes exp(logits) only for the
        # softmax denominator (E discarded); pass B recomputes logits from the
        # SBUF-resident fp8 wemb and adds the scattered 512*Delta in PSUM.
        NRES = 13                       # wemb chunks resident in SBUF
        strm = octx.enter_context(tc.tile_pool(name="strm", bufs=1))
        strm2 = octx.enter_context(tc.tile_pool(name="strm2", bufs=2))
        scrp = octx.enter_context(tc.tile_pool(name="scrp", bufs=1))
        dgp = octx.enter_context(tc.tile_pool(name="dgp", bufs=1))
        outp = octx.enter_context(tc.tile_pool(name="outp", bufs=3))
        mp = octx.enter_context(tc.tile_pool(name="mp", bufs=2, space="PSUM"))

        def wload(c):
            pool, tag = (strm, f"wemb{c}") if c < NRES else (strm2, "wemb_s")
            wflat = pool.tile([P, 4 * 4 * 512], F8, tag=tag, name=f"w{c}")
            nc.sync.dma_start(wflat[:], wembW_d[c])
            return wflat.rearrange("p (ks k v) -> p ks k v", ks=4, k=4)

        # ---- pass A: logits (fp8 DoubleRow) -> exp (discarded) + row sums ----
        wtiles = [wload(c) for c in range(NCHUNK)]
        for c in range(NCHUNK):
            subws = _subwidths(c)
            for tt in range(TT):
                ps = mp.tile([P, CHUNK], F32, tag="bigps", space="PSUM")
                for k_sub, wk in enumerate(subws):
                    for kp in range(2):
                        nc.tensor.matmul(
                            ps[:, k_sub * 512:k_sub * 512 + wk],
                            lhsT=x2T[:, 2 * kp:2 * kp + 2, tt * P:(tt + 1) * P],
                            rhs=wtiles[c][:, k_sub, 2 * kp:2 * kp + 2, :wk],
                            start=(kp == 0), stop=(kp == 1),
                            perf_mode=PM.DoubleRow)
                cw = sum(subws)
                esc = scrp.tile([P, CHUNK], BF16, tag="e_scr", name="e_scr")
                nc.scalar.activation(
                    esc[:, :cw], ps[:, :cw], AF.Exp,
                    scale=1.0 / (SW * SX), bias=lne_c[:, :1],
                    accum_out=sv_parts[:, tt * NCHUNK + c:tt * NCHUNK + c + 1])

        # ---- scalars: c_row = log(gg/sv);  Cs = cg*rr*sv/(32*gg) ----
        for tt in range(TT):
            nc.vector.reduce_sum(svs[:, tt:tt + 1],
                                 sv_parts[:, tt * NCHUNK:(tt + 1) * NCHUNK],
                                 axis=mybir.AxisListType.X)
        tmp = cpool.tile([P, TT], F32, tag="btmp")
        rtmp = cpool.tile([P, TT], F32, tag="brec")
        crow = cpool.tile([P, TT], F32, tag="crow")
        cs_t = cpool.tile([P, TT], F32, tag="cs_t")
        # svs holds 8*sv (E' = 8*exp); rr holds 1/(32*sum exp_s)
        nc.vector.reciprocal(rtmp[:], svs[:])
        nc.vector.tensor_mul(tmp[:], rtmp[:], gg[:])       # gg/(8 sv)
        nc.scalar.activation(crow[:], tmp[:], AF.Ln, scale=8.0, bias=eps_log_c[:, :1])
        nc.vector.tensor_mul(tmp[:], cg[:], rr[:])         # cg*rr_true/32
        nc.vector.tensor_mul(tmp[:], tmp[:], svs[:])       # cg*rr*sv/4
        nc.vector.reciprocal(rtmp[:], gg[:])
        nc.vector.scalar_tensor_tensor(out=cs_t[:], in0=tmp[:], scalar=0.125,
                                       in1=rtmp[:], op0=OP.mult, op1=OP.mult)

        # ---- Delta phase (compact slot grid, t-major) ----
        dgrid = dgp.tile([P, nhi, T], BF16, tag="dgrid")
        for tt in range(TT):
            tsl = slice(tt * P, (tt + 1) * P)
            psl = mp.tile([P, sp], F32, tag="bigps", space="PSUM")
            for w4 in range(sp // 512):                    # logitC (fp8 DR)
                for kp in range(2):
                    nc.tensor.matmul(
                        psl[:, w4 * 512:(w4 + 1) * 512],
                        lhsT=x2T[:, 2 * kp:2 * kp + 2, tsl],
                        rhs=wembC[:, 2 * kp:2 * kp + 2, w4 * 512:(w4 + 1) * 512],
                        start=(kp == 0), stop=(kp == 1),
                        perf_mode=PM.DoubleRow)
            ecg = scrp.tile([P, sp], BF16, tag="ecg", name="ecg")
            nc.scalar.activation(ecg[:], psl[:], AF.Exp, scale=1.0 / (SW * SX), bias=eps_log_c[:, :1])
            rec = scrp.tile([P, sp], BF16, tag="rec", name="rec")
            with nc.allow_low_precision(reason="1/E_c feeds log1p-scale delta"):
                nc.vector.reciprocal(rec[:], ecg[:])
            psm = mp.tile([P, sp], F32, tag="bigps", space="PSUM")
            for w4 in range(sp // 512):                    # merged copy mass
                for k in range(4):
                    nc.tensor.matmul(
                        psm[:, w4 * 512:(w4 + 1) * 512], lhsT=exp_c[:, k, tsl],
                        rhs=pmat[:, k, w4 * 512:(w4 + 1) * 512],
                        start=(k == 0), stop=(k == 3))
            um = scrp.tile([P, sp], F32, tag="um", name="um")
            # u = Cs*merged/E_c ; Delta = Ln(1 + u) via bias
            nc.vector.scalar_tensor_tensor(out=um[:], in0=psm[:],
                                           scalar=cs_t[:, tt:tt + 1],
                                           in1=rec[:], op0=OP.mult,
                                           op1=OP.mult)
            nc.scalar.activation(um[:], um[:], AF.Ln, bias=one_c[:, :1])
            for so in range(nhi):                          # transpose to grid
                pst = mp.tile([P, CHUNK], F32, tag="bigps", space="PSUM")
                nc.tensor.transpose(pst[:, :P], um[:, so * P:(so + 1) * P],
                                    ident_f[:])
                if so % 2 == 0:
                    nc.vector.tensor_copy(dgrid[:, so, tsl], pst[:, :P])
                else:
                    nc.scalar.copy(dgrid[:, so, tsl], pst[:, :P])

        # ---- pass B: psum = 512*logit + 512*Delta ; out = psum/512 + c_row ----
        for c in range(NCHUNK):
            if c >= NRES:
                wtiles[c] = wload(c)
            wt = wtiles[c]
            for tt in range(TT):
                subws = _subwidths(c)
                ps = mp.tile([P, CHUNK], F32, tag="bigps", space="PSUM")
                for k_sub, wk in enumerate(subws):
                    w = 4 * c + k_sub                      # vocab window index
                    po = wslot * (w % wpb)
                    hi = w // wpb
                    pslice = ps[:, k_sub * 512:k_sub * 512 + wk]
                    for kp in range(2):
                        nc.tensor.matmul(
                            pslice,
                            lhsT=x2T[:, 2 * kp:2 * kp + 2, tt * P:(tt + 1) * P],
                            rhs=wt[:, k_sub, 2 * kp:2 * kp + 2, :wk],
                            start=(kp == 0), stop=False,
                            perf_mode=PM.DoubleRow)
                    nc.tensor.matmul(
                        pslice, lhsT=dgrid[po:po + wslot, hi, tt * P:(tt + 1) * P],
                        rhs=onehot[po:po + wslot, hi, :wk],
                        start=False, stop=True, tile_position=(po, 0))
                cw = sum(subws)
                ot = outp.tile([P, CHUNK], BF16, tag="out_sb")
                if c % 4 != 3:
                    nc.vector.tensor_scalar(out=ot[:, :cw], in0=ps[:, :cw],
                                            scalar1=1.0 / (SW * SX),
                                            scalar2=crow[:, tt:tt + 1],
                                            op0=OP.mult, op1=OP.add)
                else:
                    nc.scalar.activation(ot[:, :cw], ps[:, :cw], AF.Identity,
                                         bias=crow[:, tt:tt + 1],
                                         scale=1.0 / (SW * SX))
                nc.sync.dma_start(out_r[:, tt, c * CHUNK:c * CHUNK + cw],
                                  ot[:, :cw])

    nc.compile()
    return nc


def _tile_wemb(w_emb):
    wp = np.zeros((NCHUNK * 4 * 512, D), F8NP)
    wp[:V] = (w_emb.astype(np.float32) * SW).astype(F8NP)
    # [c, ks, v, hi, lo] -> [c, lo, ks, hi, v]: chunk-major fp8 blob, one DMA
    # per 2048-wide chunk, k-pairs adjacent for DoubleRow slicing
    wt = wp.reshape(NCHUNK, 4, 512, 4, P).transpose(0, 4, 1, 3, 2)
    return np.ascontiguousarray(wt.reshape(NCHUNK, P, 4 * 4 * 512))


def _prep(inputs):
    g = {k: np.asarray(v) for k, v in inputs.items()}
    f32 = np.float32

    shared = {
        "wqT": np.ascontiguousarray((g["Wq"].T * 64).astype(F8NP)),
        "wkT": np.ascontiguousarray((g["Wk"].T * 64).astype(F8NP)),
        "wvT": np.ascontiguousarray((g["Wv"].T * 64).astype(F8NP)),
        "woT": np.ascontiguousarray((g["Wo"].T * 64).astype(F8NP)),
        "w1T": np.ascontiguousarray((g["W1"].T * 64).astype(F8NP)),
        "w2T": np.ascontiguousarray((g["W2"].T * 64).astype(F8NP)),
        "wembW": _tile_wemb(g["W_emb"]),
        "bq_c": np.ascontiguousarray(g["bq"].astype(f32).reshape(4, P).T * 64),
        "bk_c": np.ascontiguousarray(g["bk"].astype(f32).reshape(4, P).T * 64),
        "bv_row": np.tile(g["bv"].astype(f32), (P, 1)),
        "bo_tok": np.tile(g["bo"].astype(f32), (P, 1)),
        "b1_c": np.ascontiguousarray(g["b1"].astype(f32).reshape(16, P).T * 64),
        "b2_c": np.ascontiguousarray(g["b2"].astype(f32).reshape(4, P).T),
        "g1_tok": np.tile(g["ln1_g"].astype(f32), (P, 1)),
        "b1g_tok": np.tile(g["ln1_b"].astype(f32), (P, 1)),
        "g2_tok": np.tile(g["ln2_g"].astype(f32) * SX, (P, 1)),
        "b2g_tok": np.tile(g["ln2_b"].astype(f32) * SX, (P, 1)),
        "wd_diff_tok": np.tile((g["Wd"][1] - g["Wd"][0]).astype(f32), (P, 1)),
        "bddiff": np.full((P, 1), float(g["bd"][1]) - float(g["bd"][0]), f32),
        "ones_row": np.ones((1, T), BF),
    }

    cs = g["copy_seq"].astype(np.int64)          # [S, B]
    mm_ = g["mem_mask"].astype(bool)             # [B, S]
    outs = g["outs"].astype(f32)                 # [T, B, D]
    mem = g["mem"].astype(f32)                   # [S, B, D]

    maxcnt = 0
    for b in range(B):
        cnt = np.bincount(cs[:, b] // 512, minlength=NWIN).max()
        maxcnt = max(maxcnt, int(cnt))
    wslot = 32
    while wslot < maxcnt:
        wslot *= 2
    assert wslot <= P, "pathological copy_seq distribution"
    sp = 64 * wslot

    per_core = []
    for b in range(B):
        idx = cs[:, b]
        # compact sorted order: window-by-window runs; slot grid: window w
        # occupies slots [w*wslot, w*wslot + n_w)
        order = []
        s_pad = np.full(sp, -1, np.int64)
        for w in range(NWIN):
            sel = np.nonzero(idx // 512 == w)[0]
            for r, s_ in enumerate(sel):
                s_pad[w * wslot + r] = s_
            order.extend(sel.tolist())
        order = np.asarray(order, np.int64)
        assert len(order) == S
        memc = mem[order, b, :]                      # [S, D] sorted
        maskrow = np.where(mm_[b, order], NEG, 0.0).astype(f32)
        live = np.nonzero(s_pad >= 0)[0]
        vg = np.full(sp, -1, np.int64)
        vg[live] = idx[s_pad[live]]
        vcomp = idx[order]                           # vocab token per compact s
        # PM[i, jg] = 1 iff compact row i carries the same token as grid slot
        # jg (dup-merge): merged mass at every dup slot is the group total
        pm = ((vcomp[:, None] == vg[None, :]) & (vg[None, :] >= 0)).astype(f32)
        # onehot carries the 512x Delta descale; only one representative slot
        # per distinct token scatters (dups would double-apply Delta)
        oh = np.zeros((P, sp // P, 512), f32)
        seen = set()
        for jg in live:
            v = int(vg[jg])
            if v in seen:
                continue
            seen.add(v)
            w = jg // wslot
            oh[jg % P, jg // P, v - 512 * w] = 512.0
        wemb_f = g["W_emb"].astype(f32)
        wc = np.zeros((D, sp), f32)
        wc[:, live] = (wemb_f[vg[live], :] * 64.0).T
        per_core.append({
            "outsT": np.ascontiguousarray(outs[:, b, :].T.astype(F8NP)),
            "outs_tok": np.ascontiguousarray(outs[:, b, :]),
            "memT": np.ascontiguousarray(memc.T.astype(F8NP)),
            "maskrow": maskrow[None, :].astype(BF),
            "pmat": pm.astype(BF),
            "onehot": np.ascontiguousarray(oh.reshape(P, -1).astype(BF)),
            "wembC": np.ascontiguousarray(wc.astype(F8NP)),
        })
    return shared, per_core, wslot


def kernel(**inputs):
    shared, per_core, wslot = _prep(inputs)
    if wslot not in _CACHE:
        _CACHE[wslot] = _build(wslot)
    nc = _CACHE[wslot]
    in_maps = [{**shared, **pc} for pc in per_core]
    res = run_bass_kernel_spmd(nc, in_maps, core_ids=list(range(B)))
    return np.stack([np.asarray(r["out"]).astype(np.float32)
                     for r in res.results], axis=1)

